# revision 23
# baseline (speedup 1.0000x reference)
"""Trainium2 Bass kernel for nn_DetectionSampler.

Contract: kernel(**inputs) takes the FULL unsharded inputs (numpy) and
returns the FULL output tuple, matching reference():
    (det_logp_mx, des_dist_mx, px_dist_mx, mask, b1, b2, sample_logp)

Strategy:
  * The weighted random sampling (jax threefry RNG, categorical + bernoulli)
    is replicated bit-exactly on host jax-CPU; it determines the
    data-dependent sizes n, m and the sampled coordinates.
  * Descriptor gathers are host-side numpy fancy indexing (pure copies).
  * The heavy compute - three [n, m] matrices - runs on 8 NeuronCores,
    column-sharded: every core computes all n rows (padded to 128-row
    tiles) against its ceil(m/8)-column shard, so every SBUF tile has the
    full 128 partitions and output DMA blocks are contiguous in DRAM.
      - des_dist_mx: PE matmul (K=128, -2*s1^T stationary) accumulated with
        the q1/q2 rank-1 terms via one DVE scalar_tensor_tensor per PSUM
        bank, then a wide ACT sqrt.
      - px_dist_mx: ACT Square with per-partition bias (gx - x2 exact,
        matching the reference's diff-then-square form), DVE add.
      - det_logp_mx: DVE tensor_scalar broadcast add.
    The per-row-tile chains are software-pipelined with a 2-tile skew and
    the des/px d2 tiles share one combo tile so a single wide ACT sqrt
    covers both. (GPSIMD bulk elementwise is avoided: its SBUF port usage
    collapses DVE/ACT throughput.)
"""

import os
import sys
from contextlib import ExitStack

import numpy as np

for _p in ("/root/.axon_site/_ro/trn_rl_repo", "/opt/trn_rl_repo"):
    if os.path.isdir(_p) and _p not in sys.path:
        sys.path.append(_p)

import jax
import jax.numpy as jnp

import concourse.bass as bass  # noqa: F401
import concourse.tile as tile
from concourse import bacc, mybir
from concourse._compat import with_exitstack
from concourse.bass_utils import run_bass_kernel_spmd

F32 = mybir.dt.float32
AF = mybir.ActivationFunctionType

CELL_D = 8
BORDER = 16
N_CORES = 8
TILE_N = 512


# ---------------------------------------------------------------------------
# Host-side sampling (bit-exact replica of the reference, on jax CPU)
# ---------------------------------------------------------------------------

def _pixel_unshuffle(x, d):
    B, C, H, W = x.shape
    x = x.reshape(B, C, H // d, d, W // d, d)
    x = x.transpose(0, 1, 3, 5, 2, 4)
    return x.reshape(B, C * d * d, H // d, W // d)


def _cell_logits_and_xy(det):
    B, _, H, W = det.shape
    m, d = BORDER, CELL_D
    d_det = _pixel_unshuffle(det[:, :, m:H - m, m:W - m], d)
    xg, yg = jnp.meshgrid(jnp.arange(W), jnp.arange(H))
    uxy = jnp.stack([xg, yg], 0)[:, m:H - m, m:W - m]
    Hc, Wc = H - 2 * m, W - 2 * m
    uxy = jnp.broadcast_to(uxy[None], (B, 2, Hc, Wc)).reshape(B * 2, 1, Hc, Wc)
    d_xy = _pixel_unshuffle(uxy, d).reshape(B, 2, d * d, Hc // d, Wc // d)
    return d_det, d_xy


def _weighted_random_sample(det, key):
    B = det.shape[0]
    d_det, d_xy = _cell_logits_and_xy(det)
    _, dd, hc, wc = d_det.shape
    logits = d_det.transpose(0, 2, 3, 1)
    k1, k2 = jax.random.split(key)
    idxs = jax.random.categorical(k1, logits)
    log_p = jnp.take_along_axis(
        jax.nn.log_softmax(logits, -1), idxs[..., None], -1)[..., 0].reshape(-1)
    gi = jnp.broadcast_to(idxs[:, None, None], (B, 2, 1, hc, wc))
    s_xy = jnp.take_along_axis(d_xy, gi, axis=2)[:, :, 0]
    x = s_xy[:, 0].reshape(-1)
    y = s_xy[:, 1].reshape(-1)
    s_det = jnp.take_along_axis(d_det, idxs[:, None], axis=1)[:, 0].reshape(-1)
    b = jnp.repeat(jnp.arange(B), hc * wc)
    bern = (jax.random.uniform(k2, s_det.shape) < jax.nn.sigmoid(s_det)).astype(s_det.dtype)
    logp = log_p + bern * s_det - jax.nn.softplus(s_det)
    keep = bern == 1.0
    return b[keep], x[keep], y[keep], logp[keep]


def _host_sample(det1, det2):
    cpu = jax.devices("cpu")[0]
    with jax.default_device(cpu):
        d1 = jnp.asarray(det1)
        d2 = jnp.asarray(det2)
        fmin = jnp.finfo(d1.dtype).min
        d1 = jnp.nan_to_num(d1, nan=fmin)
        d2 = jnp.nan_to_num(d2, nan=fmin)
        kA, kB = jax.random.split(jax.random.key(42))
        # NOTE: unpack order (b, y, x, logp) faithfully replicates reference.
        b1, y1, x1, logp1 = _weighted_random_sample(d1, kA)
        b2, y2, x2, logp2 = _weighted_random_sample(d2, kB)
        sample_logp = logp1.sum() + logp2.sum()
    out = (b1, y1, x1, logp1, b2, y2, x2, logp2, sample_logp)
    return tuple(np.asarray(o) for o in out)


# ---------------------------------------------------------------------------
# Bass/Tile kernel: per-core [R, M] distance matrices
# ---------------------------------------------------------------------------

_KERNEL_CACHE = {}


def _build_kernel(NR, MC):
    """NR: padded row count (multiple of 128, all n rows). MC: per-core
    column shard width. Column-sharded: every tile has 128 partitions."""
    key = (NR, MC)
    if key in _KERNEL_CACHE:
        return _KERNEL_CACHE[key]

    nc = bacc.Bacc("TRN2", target_bir_lowering=False, debug=False)

    s1t = nc.dram_tensor("s1t", [128, NR], F32, kind="ExternalInput").ap()   # -2*s1^T
    s2t = nc.dram_tensor("s2t", [128, MC], F32, kind="ExternalInput").ap()   # s2^T shard
    # rowv[p, 4*t+k]: row-vector k (q1,logp1,gx,gy) for global row t*128+p
    rowv = nc.dram_tensor("rowv", [128, 4 * (NR // 128)], F32, kind="ExternalInput").ap()
    # colvr: host-replicated column vectors [q2 | logp2 | x2 | y2], each [128, MC]
    colvr = nc.dram_tensor("colvr", [128, 4 * MC], F32, kind="ExternalInput").ap()

    # fused output [des | px | det] so each row-tile is one contiguous DMA
    o_all = nc.dram_tensor("o_all", [NR, 3 * MC], F32, kind="ExternalOutput").ap()

    RT = NR // 128

    @with_exitstack
    def body(ctx: ExitStack, tc: tile.TileContext):
        const = ctx.enter_context(tc.tile_pool(name="const", bufs=1))
        psum = ctx.enter_context(tc.tile_pool(name="psum", bufs=8, space="PSUM"))
        tmp = ctx.enter_context(tc.tile_pool(name="tmp", bufs=4))
        outp = ctx.enter_context(tc.tile_pool(name="outp", bufs=4))

        # small inputs first so the px/det pipelines start immediately
        rv_sb = const.tile([128, 4 * RT], F32)
        nc.sync.dma_start(rv_sb[:], rowv[:])

        # x2/y2 land in their own tile so the first ACT squares only wait
        # for half the replicated-vector bytes
        cv_xy = const.tile([128, 2 * MC], F32)
        nc.sync.dma_start(cv_xy[:], colvr[:, 2 * MC:])
        x2r, y2r = cv_xy[:, :MC], cv_xy[:, MC:]
        cv_q = const.tile([128, 2 * MC], F32)
        nc.sync.dma_start(cv_q[:], colvr[:, :2 * MC])
        q2r, lp2r = cv_q[:, :MC], cv_q[:, MC:]

        s2_sb = const.tile([128, MC], F32)
        nc.sync.dma_start(s2_sb[:], s2t[:])
        # s1^T in 512-col chunk tiles: row-tile i's matmul only depends on
        # chunk i//4 instead of the whole 2.2MB load
        s1_chunks = []
        for c0 in range(0, NR, 512):
            w = min(512, NR - c0)
            t = const.tile([128, w], F32, tag=f"s1c{c0 // 512}")
            nc.sync.dma_start(t[:], s1t[:, c0:c0 + w])
            s1_chunks.append(t)

        def biases(i):
            return tuple(rv_sb[:, 4 * i + k:4 * i + k + 1] for k in range(4))

        # Software-pipelined with a 2-row-tile skew; des d2 and px d2 land in
        # one combined [128, 2*MC] tile so a single wide ACT sqrt finishes
        # both (per-op overhead is ~45% of a 540-wide ACT op).
        SKEW = 1
        combo_t = [None] * RT
        for ii in range(RT + SKEW):
            if ii < RT:
                i = ii
                r0 = i * 128
                q1b, lp1b, gxb, gyb = biases(i)

                combo = tmp.tile([128, 3 * MC], F32, tag="combo")

                # px squares (only need reps + biases); roughly half the sqx
                # ops run on DVE (sub + self-mult) to balance ACT vs DVE
                sqx = tmp.tile([128, MC], F32, tag="sqx")
                if i % 4 == 0:
                    dx = tmp.tile([128, MC], F32, tag="dx")
                    nc.vector.tensor_scalar_sub(dx[:], x2r[:], gxb)
                    nc.vector.tensor_mul(sqx[:], dx[:], dx[:])
                else:
                    nc.scalar.activation(sqx[:], x2r[:], AF.Square, bias=gxb, scale=-1.0)
                sqy = tmp.tile([128, MC], F32, tag="sqy")
                nc.scalar.activation(sqy[:], y2r[:], AF.Square, bias=gyb, scale=-1.0)

                # des d2 = (psum + q1) + q2 in one DVE op per PSUM subtile
                for s0 in range(0, MC, TILE_N):
                    s1_ = min(s0 + TILE_N, MC)
                    Ws = s1_ - s0
                    ps = psum.tile([128, TILE_N], F32, tag="ps")
                    s1c = s1_chunks[i // 4]
                    co = (i % 4) * 128
                    nc.tensor.matmul(ps[:, :Ws], s1c[:, co:co + 128], s2_sb[:, s0:s1_],
                                     start=True, stop=True)
                    nc.vector.scalar_tensor_tensor(
                        combo[:, s0:s1_], ps[:, :Ws], q1b, q2r[:, s0:s1_],
                        op0=mybir.AluOpType.add, op1=mybir.AluOpType.add)

                # det on DVE (GPSIMD tensor ops wreck SBUF-port bandwidth
                # for every other engine - measured 3x kernel slowdown),
                # written into the combo tail so one DMA covers all three
                nc.vector.tensor_scalar_add(combo[:, 2 * MC:], lp2r[:], lp1b)
                nc.vector.tensor_add(combo[:, MC:2 * MC], sqx[:], sqy[:])
                combo_t[i] = combo

            if ii >= SKEW:
                i = ii - SKEW
                r0 = i * 128
                combo = combo_t[i]
                nc.scalar.activation(combo[:, :2 * MC], combo[:, :2 * MC], AF.Sqrt)
                nc.sync.dma_start(o_all[r0:r0 + 128, :], combo[:])
                combo_t[i] = None

    with tile.TileContext(nc) as tc:
        body(tc)
    nc.compile()
    _KERNEL_CACHE[key] = nc
    return nc


# ---------------------------------------------------------------------------
# Device run orchestration
# ---------------------------------------------------------------------------

def _run_device(s1, s2, lp1, lp2, gx, gy, x2f, y2f, trace=False):
    n = s1.shape[0]
    m = s2.shape[0]
    NR = ((n + 127) // 128) * 128
    MC = (m + N_CORES - 1) // N_CORES
    MT = N_CORES * MC

    q1 = np.einsum('nd,nd->n', s1, s1).astype(np.float32)
    q2 = np.einsum('md,md->m', s2, s2).astype(np.float32)

    rowv_flat = np.zeros((NR, 4), np.float32)
    rowv_flat[:n, 0] = q1
    rowv_flat[:n, 1] = lp1
    rowv_flat[:n, 2] = gx
    rowv_flat[:n, 3] = gy
    # device layout: [128, 4*RT], rowv[p, 4*t+k] = rowv_flat[t*128+p, k]
    rowv = np.ascontiguousarray(
        rowv_flat.reshape(NR // 128, 128, 4).transpose(1, 0, 2).reshape(128, -1))
    s1t = np.zeros((128, NR), np.float32)
    s1t[:, :n] = (-2.0 * s1).T
    colv_full = np.zeros((4, MT), np.float32)
    colv_full[0, :m] = q2
    colv_full[1, :m] = lp2
    colv_full[2, :m] = x2f
    colv_full[3, :m] = y2f
    colvr_full = np.broadcast_to(colv_full.reshape(4, N_CORES, MC), (128, 4, N_CORES, MC))
    s2t_full = np.zeros((128, MT), np.float32)
    s2t_full[:, :m] = s2.T

    nc = _build_kernel(NR, MC)
    in_maps = []
    for c in range(N_CORES):
        sl = slice(c * MC, (c + 1) * MC)
        in_maps.append({
            "s1t": s1t,
            "s2t": np.ascontiguousarray(s2t_full[:, sl]),
            "rowv": rowv,
            "colvr": np.ascontiguousarray(colvr_full[:, :, c].reshape(128, 4 * MC)),
        })
    res = run_bass_kernel_spmd(nc, in_maps, list(range(N_CORES)), trace=trace)
    des = np.concatenate(
        [res.results[c]["o_all"][:, 0 * MC:1 * MC] for c in range(N_CORES)], axis=1)[:n, :m]
    px = np.concatenate(
        [res.results[c]["o_all"][:, 1 * MC:2 * MC] for c in range(N_CORES)], axis=1)[:n, :m]
    det = np.concatenate(
        [res.results[c]["o_all"][:, 2 * MC:3 * MC] for c in range(N_CORES)], axis=1)[:n, :m]
    return det, des, px, res


def _run_device_resilient(s1, s2, lp1, lp2, gx, gy, x2f, y2f, trace=False):
    """The neuron device occasionally reports NRT_EXEC_UNIT_UNRECOVERABLE on a
    fresh NEFF load; once that happens the in-process PJRT client stays
    poisoned, but a fresh process recovers. Try in-process, then fall back to
    subprocess attempts."""
    try:
        return _run_device(s1, s2, lp1, lp2, gx, gy, x2f, y2f, trace=trace)
    except Exception as first_err:  # noqa: BLE001
        sys.stderr.write(f"kernel: in-process device run failed ({first_err!r}); "
                         "retrying in a fresh subprocess\n")
    import pickle
    import subprocess
    import tempfile
    payload = dict(s1=s1, s2=s2, lp1=lp1, lp2=lp2, gx=gx, gy=gy, x2f=x2f, y2f=y2f)
    kdir = os.path.dirname(os.path.abspath(__file__))
    last_err = None
    for _attempt in range(2):
        with tempfile.TemporaryDirectory() as td:
            inp = os.path.join(td, "in.pkl")
            outp = os.path.join(td, "out.pkl")
            with open(inp, "wb") as f:
                pickle.dump(payload, f)
            script = (
                "import pickle, sys\n"
                f"sys.path.insert(0, {kdir!r})\n"
                "import kernel\n"
                f"p = pickle.load(open({inp!r}, 'rb'))\n"
                "det, des, px, _ = kernel._run_device(**p, trace=False)\n"
                f"pickle.dump((det, des, px), open({outp!r}, 'wb'))\n"
            )
            try:
                subprocess.run([sys.executable, "-c", script], check=True,
                               timeout=1800)
                with open(outp, "rb") as f:
                    det, des, px = pickle.load(f)
                return det, des, px, None
            except Exception as e:  # noqa: BLE001
                last_err = e
    raise last_err


def kernel(des1, det1, des2, det2, aflow, _trace=False, _return_res=False):
    des1 = np.asarray(des1)
    des2 = np.asarray(des2)
    aflow = np.asarray(aflow)

    (b1, y1, x1, logp1, b2, y2, x2, logp2, sample_logp) = _host_sample(det1, det2)

    s1 = des1[b1, :, y1, x1]                    # [n, 128]
    s2 = des2[b2, :, y2, x2]                    # [m, 128]
    xy2_gt = aflow[b1, :, y1, x1].T             # [2, n]
    H, W = aflow.shape[2], aflow.shape[3]
    mask = (xy2_gt[0] >= 0) & (xy2_gt[1] >= 0) & (xy2_gt[0] < W) & (xy2_gt[1] < H)

    x2f = x2.astype(np.float32)
    y2f = y2.astype(np.float32)

    det_logp_mx, des_dist_mx, px_dist_mx, res = _run_device_resilient(
        s1, s2, logp1, logp2, xy2_gt[0], xy2_gt[1], x2f, y2f, trace=_trace)

    out = (det_logp_mx, des_dist_mx, px_dist_mx, mask,
           b1.astype(np.int32), b2.astype(np.int32), np.float32(sample_logp))
    if _return_res:
        return out, res
    return out


# revision 24
# speedup vs baseline: 1.1889x; 1.1889x over previous
"""Trainium2 Bass kernel for nn_DetectionSampler.

Contract: kernel(**inputs) takes the FULL unsharded inputs (numpy) and
returns the FULL output tuple, matching reference():
    (det_logp_mx, des_dist_mx, px_dist_mx, mask, b1, b2, sample_logp)

Strategy:
  * The weighted random sampling (jax threefry RNG, categorical + bernoulli)
    is replicated bit-exactly on host jax-CPU; it determines the
    data-dependent sizes n, m and the sampled coordinates.
  * Descriptor gathers are host-side numpy fancy indexing (pure copies).
  * The heavy compute - three [n, m] matrices - runs on 8 NeuronCores,
    column-sharded: every core computes all n rows (padded to 128-row
    tiles) against its ceil(m/8)-column shard, so every SBUF tile has the
    full 128 partitions and output DMA blocks are contiguous in DRAM.
      - des_dist_mx: PE matmul (K=128, -2*s1^T stationary) accumulated with
        the q1/q2 rank-1 terms via one DVE scalar_tensor_tensor per PSUM
        bank, then a wide ACT sqrt.
      - px_dist_mx: ACT Square with per-partition bias (gx - x2 exact,
        matching the reference's diff-then-square form), DVE add.
      - det_logp_mx: DVE tensor_scalar broadcast add.
    The per-row-tile chains are software-pipelined with a 2-tile skew and
    the des/px d2 tiles share one combo tile so a single wide ACT sqrt
    covers both. (GPSIMD bulk elementwise is avoided: its SBUF port usage
    collapses DVE/ACT throughput.)
"""

import os
import sys
from contextlib import ExitStack

import numpy as np

for _p in ("/root/.axon_site/_ro/trn_rl_repo", "/opt/trn_rl_repo"):
    if os.path.isdir(_p) and _p not in sys.path:
        sys.path.append(_p)

import jax
import jax.numpy as jnp

import concourse.bass as bass  # noqa: F401
import concourse.tile as tile
from concourse import bacc, mybir
from concourse._compat import with_exitstack
from concourse.bass_utils import run_bass_kernel_spmd

F32 = mybir.dt.float32
AF = mybir.ActivationFunctionType

CELL_D = 8
BORDER = 16
N_CORES = 8
TILE_N = 512


# ---------------------------------------------------------------------------
# Host-side sampling (bit-exact replica of the reference, on jax CPU)
# ---------------------------------------------------------------------------

def _pixel_unshuffle(x, d):
    B, C, H, W = x.shape
    x = x.reshape(B, C, H // d, d, W // d, d)
    x = x.transpose(0, 1, 3, 5, 2, 4)
    return x.reshape(B, C * d * d, H // d, W // d)


def _cell_logits_and_xy(det):
    B, _, H, W = det.shape
    m, d = BORDER, CELL_D
    d_det = _pixel_unshuffle(det[:, :, m:H - m, m:W - m], d)
    xg, yg = jnp.meshgrid(jnp.arange(W), jnp.arange(H))
    uxy = jnp.stack([xg, yg], 0)[:, m:H - m, m:W - m]
    Hc, Wc = H - 2 * m, W - 2 * m
    uxy = jnp.broadcast_to(uxy[None], (B, 2, Hc, Wc)).reshape(B * 2, 1, Hc, Wc)
    d_xy = _pixel_unshuffle(uxy, d).reshape(B, 2, d * d, Hc // d, Wc // d)
    return d_det, d_xy


def _weighted_random_sample(det, key):
    B = det.shape[0]
    d_det, d_xy = _cell_logits_and_xy(det)
    _, dd, hc, wc = d_det.shape
    logits = d_det.transpose(0, 2, 3, 1)
    k1, k2 = jax.random.split(key)
    idxs = jax.random.categorical(k1, logits)
    log_p = jnp.take_along_axis(
        jax.nn.log_softmax(logits, -1), idxs[..., None], -1)[..., 0].reshape(-1)
    gi = jnp.broadcast_to(idxs[:, None, None], (B, 2, 1, hc, wc))
    s_xy = jnp.take_along_axis(d_xy, gi, axis=2)[:, :, 0]
    x = s_xy[:, 0].reshape(-1)
    y = s_xy[:, 1].reshape(-1)
    s_det = jnp.take_along_axis(d_det, idxs[:, None], axis=1)[:, 0].reshape(-1)
    b = jnp.repeat(jnp.arange(B), hc * wc)
    bern = (jax.random.uniform(k2, s_det.shape) < jax.nn.sigmoid(s_det)).astype(s_det.dtype)
    logp = log_p + bern * s_det - jax.nn.softplus(s_det)
    keep = bern == 1.0
    return b[keep], x[keep], y[keep], logp[keep]


def _host_sample(det1, det2):
    cpu = jax.devices("cpu")[0]
    with jax.default_device(cpu):
        d1 = jnp.asarray(det1)
        d2 = jnp.asarray(det2)
        fmin = jnp.finfo(d1.dtype).min
        d1 = jnp.nan_to_num(d1, nan=fmin)
        d2 = jnp.nan_to_num(d2, nan=fmin)
        kA, kB = jax.random.split(jax.random.key(42))
        # NOTE: unpack order (b, y, x, logp) faithfully replicates reference.
        b1, y1, x1, logp1 = _weighted_random_sample(d1, kA)
        b2, y2, x2, logp2 = _weighted_random_sample(d2, kB)
        sample_logp = logp1.sum() + logp2.sum()
    out = (b1, y1, x1, logp1, b2, y2, x2, logp2, sample_logp)
    return tuple(np.asarray(o) for o in out)


# ---------------------------------------------------------------------------
# Bass/Tile kernel: per-core [R, M] distance matrices
# ---------------------------------------------------------------------------

_KERNEL_CACHE = {}


def _build_kernel(NR, MC):
    """NR: padded row count (multiple of 128, all n rows). MC: per-core
    column shard width. Column-sharded: every tile has 128 partitions."""
    key = (NR, MC)
    if key in _KERNEL_CACHE:
        return _KERNEL_CACHE[key]

    nc = bacc.Bacc("TRN2", target_bir_lowering=False, debug=False)

    s1t = nc.dram_tensor("s1t", [128, NR], F32, kind="ExternalInput").ap()   # -2*s1^T
    s2t = nc.dram_tensor("s2t", [128, MC], F32, kind="ExternalInput").ap()   # s2^T shard
    # rowv[p, 4*t+k]: row-vector k (q1,logp1,gx,gy) for global row t*128+p
    rowv = nc.dram_tensor("rowv", [128, 4 * (NR // 128)], F32, kind="ExternalInput").ap()
    # colvr: host-replicated column vectors [q2 | logp2 | x2 | y2], each [128, MC]
    colvr = nc.dram_tensor("colvr", [128, 4 * MC], F32, kind="ExternalInput").ap()

    # fused output [des | px | det] so each row-tile is one contiguous DMA
    o_all = nc.dram_tensor("o_all", [NR, 3 * MC], F32, kind="ExternalOutput").ap()

    RT = NR // 128

    @with_exitstack
    def body(ctx: ExitStack, tc: tile.TileContext):
        const = ctx.enter_context(tc.tile_pool(name="const", bufs=1))
        psum = ctx.enter_context(tc.tile_pool(name="psum", bufs=8, space="PSUM"))
        tmp = ctx.enter_context(tc.tile_pool(name="tmp", bufs=4))
        outp = ctx.enter_context(tc.tile_pool(name="outp", bufs=4))

        # small inputs first so the px/det pipelines start immediately
        rv_sb = const.tile([128, 4 * RT], F32)
        nc.sync.dma_start(rv_sb[:], rowv[:])

        # x2/y2 land in their own tile so the first ACT squares only wait
        # for half the replicated-vector bytes
        cv_xy = const.tile([128, 2 * MC], F32)
        nc.sync.dma_start(cv_xy[:], colvr[:, 2 * MC:])
        x2r, y2r = cv_xy[:, :MC], cv_xy[:, MC:]
        cv_q = const.tile([128, 2 * MC], F32)
        nc.sync.dma_start(cv_q[:], colvr[:, :2 * MC])
        q2r, lp2r = cv_q[:, :MC], cv_q[:, MC:]

        s2_sb = const.tile([128, MC], F32)
        nc.sync.dma_start(s2_sb[:], s2t[:])
        # s1^T in 512-col chunk tiles: row-tile i's matmul only depends on
        # chunk i//4 instead of the whole 2.2MB load
        s1_chunks = []
        for c0 in range(0, NR, 512):
            w = min(512, NR - c0)
            t = const.tile([128, w], F32, tag=f"s1c{c0 // 512}")
            nc.sync.dma_start(t[:], s1t[:, c0:c0 + w])
            s1_chunks.append(t)

        def biases(i):
            return tuple(rv_sb[:, 4 * i + k:4 * i + k + 1] for k in range(4))

        # Software-pipelined with a 2-row-tile skew; des d2 and px d2 land in
        # one combined [128, 2*MC] tile so a single wide ACT sqrt finishes
        # both (per-op overhead is ~45% of a 540-wide ACT op).
        SKEW = 2
        combo_t = [None] * RT
        for ii in range(RT + SKEW):
            if ii < RT:
                i = ii
                r0 = i * 128
                q1b, lp1b, gxb, gyb = biases(i)

                combo = tmp.tile([128, 3 * MC], F32, tag="combo")

                # px squares (only need reps + biases); roughly half the sqx
                # ops run on DVE (sub + self-mult) to balance ACT vs DVE
                sqx = tmp.tile([128, MC], F32, tag="sqx")
                if i % 4 == 0:
                    dx = tmp.tile([128, MC], F32, tag="dx")
                    nc.vector.tensor_scalar_sub(dx[:], x2r[:], gxb)
                    nc.vector.tensor_mul(sqx[:], dx[:], dx[:])
                else:
                    nc.scalar.activation(sqx[:], x2r[:], AF.Square, bias=gxb, scale=-1.0)
                sqy = tmp.tile([128, MC], F32, tag="sqy")
                nc.scalar.activation(sqy[:], y2r[:], AF.Square, bias=gyb, scale=-1.0)

                # des d2 = (psum + q1) + q2 in one DVE op per PSUM subtile
                for s0 in range(0, MC, TILE_N):
                    s1_ = min(s0 + TILE_N, MC)
                    Ws = s1_ - s0
                    ps = psum.tile([128, TILE_N], F32, tag="ps")
                    s1c = s1_chunks[i // 4]
                    co = (i % 4) * 128
                    nc.tensor.matmul(ps[:, :Ws], s1c[:, co:co + 128], s2_sb[:, s0:s1_],
                                     start=True, stop=True)
                    nc.vector.scalar_tensor_tensor(
                        combo[:, s0:s1_], ps[:, :Ws], q1b, q2r[:, s0:s1_],
                        op0=mybir.AluOpType.add, op1=mybir.AluOpType.add)

                # det on DVE (GPSIMD tensor ops wreck SBUF-port bandwidth
                # for every other engine - measured 3x kernel slowdown),
                # written into the combo tail so one DMA covers all three
                nc.vector.tensor_scalar_add(combo[:, 2 * MC:], lp2r[:], lp1b)
                nc.vector.tensor_add(combo[:, MC:2 * MC], sqx[:], sqy[:])
                combo_t[i] = combo

            if ii >= SKEW:
                i = ii - SKEW
                r0 = i * 128
                combo = combo_t[i]
                nc.scalar.activation(combo[:, :2 * MC], combo[:, :2 * MC], AF.Sqrt)
                nc.sync.dma_start(o_all[r0:r0 + 128, :], combo[:])
                combo_t[i] = None

    with tile.TileContext(nc) as tc:
        body(tc)
    nc.compile()
    _KERNEL_CACHE[key] = nc
    return nc


# ---------------------------------------------------------------------------
# Device run orchestration
# ---------------------------------------------------------------------------

def _run_device(s1, s2, lp1, lp2, gx, gy, x2f, y2f, trace=False):
    n = s1.shape[0]
    m = s2.shape[0]
    NR = ((n + 127) // 128) * 128
    MC = (m + N_CORES - 1) // N_CORES
    MT = N_CORES * MC

    q1 = np.einsum('nd,nd->n', s1, s1).astype(np.float32)
    q2 = np.einsum('md,md->m', s2, s2).astype(np.float32)

    rowv_flat = np.zeros((NR, 4), np.float32)
    rowv_flat[:n, 0] = q1
    rowv_flat[:n, 1] = lp1
    rowv_flat[:n, 2] = gx
    rowv_flat[:n, 3] = gy
    # device layout: [128, 4*RT], rowv[p, 4*t+k] = rowv_flat[t*128+p, k]
    rowv = np.ascontiguousarray(
        rowv_flat.reshape(NR // 128, 128, 4).transpose(1, 0, 2).reshape(128, -1))
    s1t = np.zeros((128, NR), np.float32)
    s1t[:, :n] = (-2.0 * s1).T
    colv_full = np.zeros((4, MT), np.float32)
    colv_full[0, :m] = q2
    colv_full[1, :m] = lp2
    colv_full[2, :m] = x2f
    colv_full[3, :m] = y2f
    colvr_full = np.broadcast_to(colv_full.reshape(4, N_CORES, MC), (128, 4, N_CORES, MC))
    s2t_full = np.zeros((128, MT), np.float32)
    s2t_full[:, :m] = s2.T

    nc = _build_kernel(NR, MC)
    in_maps = []
    for c in range(N_CORES):
        sl = slice(c * MC, (c + 1) * MC)
        in_maps.append({
            "s1t": s1t,
            "s2t": np.ascontiguousarray(s2t_full[:, sl]),
            "rowv": rowv,
            "colvr": np.ascontiguousarray(colvr_full[:, :, c].reshape(128, 4 * MC)),
        })
    res = run_bass_kernel_spmd(nc, in_maps, list(range(N_CORES)), trace=trace)
    des = np.concatenate(
        [res.results[c]["o_all"][:, 0 * MC:1 * MC] for c in range(N_CORES)], axis=1)[:n, :m]
    px = np.concatenate(
        [res.results[c]["o_all"][:, 1 * MC:2 * MC] for c in range(N_CORES)], axis=1)[:n, :m]
    det = np.concatenate(
        [res.results[c]["o_all"][:, 2 * MC:3 * MC] for c in range(N_CORES)], axis=1)[:n, :m]
    return det, des, px, res


def _run_device_resilient(s1, s2, lp1, lp2, gx, gy, x2f, y2f, trace=False):
    """The neuron device occasionally reports NRT_EXEC_UNIT_UNRECOVERABLE on a
    fresh NEFF load; once that happens the in-process PJRT client stays
    poisoned, but a fresh process recovers. Try in-process, then fall back to
    subprocess attempts."""
    try:
        return _run_device(s1, s2, lp1, lp2, gx, gy, x2f, y2f, trace=trace)
    except Exception as first_err:  # noqa: BLE001
        sys.stderr.write(f"kernel: in-process device run failed ({first_err!r}); "
                         "retrying in a fresh subprocess\n")
    import pickle
    import subprocess
    import tempfile
    payload = dict(s1=s1, s2=s2, lp1=lp1, lp2=lp2, gx=gx, gy=gy, x2f=x2f, y2f=y2f)
    kdir = os.path.dirname(os.path.abspath(__file__))
    last_err = None
    for _attempt in range(2):
        with tempfile.TemporaryDirectory() as td:
            inp = os.path.join(td, "in.pkl")
            outp = os.path.join(td, "out.pkl")
            with open(inp, "wb") as f:
                pickle.dump(payload, f)
            script = (
                "import pickle, sys\n"
                f"sys.path.insert(0, {kdir!r})\n"
                "import kernel\n"
                f"p = pickle.load(open({inp!r}, 'rb'))\n"
                "det, des, px, _ = kernel._run_device(**p, trace=False)\n"
                f"pickle.dump((det, des, px), open({outp!r}, 'wb'))\n"
            )
            try:
                subprocess.run([sys.executable, "-c", script], check=True,
                               timeout=1800)
                with open(outp, "rb") as f:
                    det, des, px = pickle.load(f)
                return det, des, px, None
            except Exception as e:  # noqa: BLE001
                last_err = e
    raise last_err


def kernel(des1, det1, des2, det2, aflow, _trace=False, _return_res=False):
    des1 = np.asarray(des1)
    des2 = np.asarray(des2)
    aflow = np.asarray(aflow)

    (b1, y1, x1, logp1, b2, y2, x2, logp2, sample_logp) = _host_sample(det1, det2)

    s1 = des1[b1, :, y1, x1]                    # [n, 128]
    s2 = des2[b2, :, y2, x2]                    # [m, 128]
    xy2_gt = aflow[b1, :, y1, x1].T             # [2, n]
    H, W = aflow.shape[2], aflow.shape[3]
    mask = (xy2_gt[0] >= 0) & (xy2_gt[1] >= 0) & (xy2_gt[0] < W) & (xy2_gt[1] < H)

    x2f = x2.astype(np.float32)
    y2f = y2.astype(np.float32)

    det_logp_mx, des_dist_mx, px_dist_mx, res = _run_device_resilient(
        s1, s2, logp1, logp2, xy2_gt[0], xy2_gt[1], x2f, y2f, trace=_trace)

    out = (det_logp_mx, des_dist_mx, px_dist_mx, mask,
           b1.astype(np.int32), b2.astype(np.int32), np.float32(sample_logp))
    if _return_res:
        return out, res
    return out
